# revision 8
# baseline (speedup 1.0000x reference)
"""Trainium2 Bass kernel for location-sensitive attention.

alpha = softmax(w_score . tanh(enc @ W_enc + b_enc + h @ W_dec + conv(prev_alpha) @ W_c2s)) * mask

Sharding: data-parallel over batch B=32 across 8 cores (4 batches/core).
All weights replicated. Full inputs in, full output out.

Per-core dataflow (T=2000, K=1024, A=512, batches=4):
  - enc tiles DMA'd naturally as [t<=128, 1024] (contiguous rows).
  - TensorE transpose-mode flips each [t,128k] block into PSUM as [128k, t];
    ACT/DVE copy assembles encT [128k, t] in SBUF.
  - Regular matmuls (float32r: full-rate fp32 data) accumulate in PSUM [a128, t512]:
      8 chunks of W_enc.T-contraction + 1 conv matmul (Hankel view of padded
      alpha against M = W_conv.T @ W_c2s, a rank-100 contraction).
  - ACT applies tanh PSUM->SBUF with per-partition bias = dec_e[b] + b_enc
    (computed transposed on-device); TensorE contracts with w_score into
    PSUM e[1, t]; ACT applies exp (softmax max-subtraction is skipped:
    |e| <= ||w_score||_1 ~ 16, safely inside fp32 exp range; alpha invariant).
  - Rows packed [4, T] via a DRAM bounce; DVE masked sum + reciprocal + scale.
"""

import os
import sys
import numpy as np
import dataclasses

for _p in ("/opt/trn_rl_repo", "/root/.axon_site/_ro/trn_rl_repo"):
    if os.path.isdir(_p) and _p not in sys.path:
        sys.path.append(_p)

import concourse.bass as bass
import concourse.bacc as bacc
import concourse.mybir as mybir
from concourse import tile

B, T, ENC2, DEC, ATTN = 32, 2000, 1024, 512, 512
NK, KW, PAD = 10, 100, 50
NCORES = 8
BPC = B // NCORES  # batches per core
TP = T + KW  # padded alpha length (50 + 2000 + 50)

F32 = mybir.dt.float32
F32R = mybir.dt.float32r
AF = mybir.ActivationFunctionType

KCH = ENC2 // 128  # 8 contraction chunks
ACH = ATTN // 128  # 4 a-chunks
T_TILES = [(0, 512), (512, 512), (1024, 512), (1536, 464)]


def _subchunks(tt):
    subs = []
    j0 = 0
    while j0 < tt:
        subs.append((j0, min(128, tt - j0)))
        j0 += 128
    return subs


def r32(ap):
    return ap.bitcast(F32R)


def build_nc(n_batches=None, n_ttiles=None, do_main=True, do_tail=True):
    if n_batches is None:
        n_batches = int(os.environ.get("K_BATCHES", BPC))
    if n_ttiles is None:
        n_ttiles = int(os.environ.get("K_TTILES", len(T_TILES)))
    do_main = do_main and os.environ.get("K_MAIN", "1") == "1"
    do_tail = do_tail and os.environ.get("K_TAIL", "1") == "1"
    nc = bacc.Bacc(None, target_bir_lowering=False)

    enc = nc.declare_dram_parameter("enc", [BPC, T, ENC2], F32, isOutput=False)
    apad = nc.declare_dram_parameter("apad", [BPC, TP], F32, isOutput=False)
    mask = nc.declare_dram_parameter("mask", [BPC, T], F32, isOutput=False)
    ht = nc.declare_dram_parameter("hT", [DEC, BPC], F32, isOutput=False)
    wconv = nc.declare_dram_parameter("wconv", [NK, KW], F32, isOutput=False)
    wc2s = nc.declare_dram_parameter("wc2s", [NK, ATTN], F32, isOutput=False)
    wenc = nc.declare_dram_parameter("wenc", [ENC2, ATTN], F32, isOutput=False)
    bencT = nc.declare_dram_parameter("bencT", [128, ACH], F32, isOutput=False)
    wdec = nc.declare_dram_parameter("wdec", [DEC, ATTN], F32, isOutput=False)
    wsc = nc.declare_dram_parameter("wsc", [128, ACH], F32, isOutput=False)
    ident = nc.declare_dram_parameter("ident", [128, 128], F32, isOutput=False)
    out = nc.declare_dram_parameter("out", [BPC, T], F32, isOutput=True)

    e_scr = nc.dram_tensor("e_scr", [BPC, T], F32)

    with tile.TileContext(nc) as tc:
        with (
            tc.tile_pool(name="const", bufs=1) as cpool,
            tc.tile_pool(name="nat", bufs=8) as nat_pool,
            tc.tile_pool(name="encT", bufs=2) as encT_pool,
            tc.tile_pool(name="th", bufs=4) as th_pool,
            tc.tile_pool(name="eb", bufs=2) as eb_pool,
            tc.tile_pool(name="tail", bufs=1) as tail_pool,
            tc.tile_pool(name="ptr", bufs=3, space="PSUM") as ptr_pool,
            tc.tile_pool(name="pacc", bufs=2, space="PSUM") as pacc_pool,
            tc.tile_pool(name="pe", bufs=2, space="PSUM") as pe_pool,
        ):
            # ---- constants / weights into SBUF ----
            W_sb = cpool.tile([128, KCH * ATTN], F32R)  # [128, 4096]
            for ki in range(KCH):
                nc.gpsimd.dma_start(
                    W_sb[:, ki * ATTN : (ki + 1) * ATTN],
                    wenc[ki * 128 : (ki + 1) * 128, :],
                )
            wc_sb = cpool.tile([NK, KW], F32)
            nc.sync.dma_start(wc_sb[:, :], wconv[:, :])
            wcs_sb = cpool.tile([NK, ATTN], F32)
            nc.sync.dma_start(wcs_sb[:, :], wc2s[:, :])
            ht_sb = cpool.tile([128, 4 * BPC], F32)
            for c in range(4):
                nc.sync.dma_start(
                    ht_sb[:, c * BPC : (c + 1) * BPC],
                    ht[c * 128 : (c + 1) * 128, :],
                )
            wd_sb = cpool.tile([128, 4 * ATTN], F32)
            for c in range(4):
                nc.sync.dma_start(
                    wd_sb[:, c * ATTN : (c + 1) * ATTN],
                    wdec[c * 128 : (c + 1) * 128, :],
                )
            be_sb = cpool.tile([128, ACH], F32)
            nc.sync.dma_start(be_sb[:, :], bencT[:, :])
            msk_sb = cpool.tile([BPC, T], F32)
            nc.sync.dma_start(msk_sb[:, :], mask[:, :])
            ws_sb = cpool.tile([128, ACH], F32R)
            nc.gpsimd.dma_start(ws_sb[:, :], wsc[:, :])
            id_sb = cpool.tile([128, 128], F32)
            nc.sync.dma_start(id_sb[:, :], ident[:, :])

            # M = wconv.T @ wc2s  [100, 512] ; decbe [128, ACH*BPC]:
            #   decbe[p, ac*BPC+b] = sum_d h[b,d] wdec[d, ac*128+p] + b_enc[ac*128+p]
            M_sb = cpool.tile([KW, ATTN], F32R)
            decbe = cpool.tile([128, ACH * BPC], F32)
            # H: [100, BPC*2000] Hankel(alpha_pad)
            H = cpool.tile([KW, BPC * T], F32R)

            with tc.tile_pool(name="psetup", bufs=1, space="PSUM") as spool:
                m_ps = spool.tile([KW, ATTN], F32, tag="s")
                nc.tensor.matmul(
                    m_ps[:, :], wc_sb[:, :], wcs_sb[:, :], start=True, stop=True
                )
                nc.scalar.copy(M_sb[:, :], m_ps[:, :])
                dec_ps = spool.tile([128, ACH * BPC], F32, tag="s")
                for ac in range(ACH):
                    for c in range(4):
                        nc.tensor.matmul(
                            dec_ps[:, ac * BPC : (ac + 1) * BPC],
                            wd_sb[:, c * ATTN + ac * 128 : c * ATTN + (ac + 1) * 128],
                            ht_sb[:, c * BPC : (c + 1) * BPC],
                            start=(c == 0),
                            stop=(c == 3),
                        )
                for ac in range(ACH):
                    nc.scalar.activation(
                        decbe[:, ac * BPC : (ac + 1) * BPC],
                        dec_ps[:, ac * BPC : (ac + 1) * BPC],
                        AF.Identity,
                        bias=be_sb[:, ac : ac + 1],
                    )

            for b in range(BPC):
                hank = dataclasses.replace(
                    apad[b : b + 1, :], ap=[[1, KW], [1, T]]
                )
                nc.gpsimd.dma_start(H[0:KW, b * T : (b + 1) * T], hank)

            # ---- main loop ----
            for b in range(n_batches if do_main else 0):
                for t0, tt in T_TILES[:n_ttiles]:
                    subs = _subchunks(tt)
                    nats = []
                    for j0, tj in subs:
                        natt = nat_pool.tile([128, ENC2], F32)
                        nc.sync.dma_start(
                            natt[0:tj, :], enc[b, t0 + j0 : t0 + j0 + tj, :]
                        )
                        nats.append(natt)
                    encT = encT_pool.tile([128, KCH * 512], F32R)
                    for ki in range(KCH):
                        ptr = ptr_pool.tile([128, 512], F32)
                        for idx, (j0, tj) in enumerate(subs):
                            nc.tensor.transpose(
                                ptr[:, j0 : j0 + tj],
                                nats[idx][0:tj, ki * 128 : (ki + 1) * 128],
                                id_sb[0:tj, 0:tj],
                            )
                        if ki % 2 == 0:
                            nc.scalar.copy(
                                encT[:, ki * 512 : ki * 512 + tt], ptr[:, 0:tt]
                            )
                        else:
                            nc.vector.tensor_copy(
                                encT[:, ki * 512 : ki * 512 + tt], ptr[:, 0:tt]
                            )
                    pe_ps = pe_pool.tile([1, 512], F32)
                    for ac in range(ACH):
                        pacc = pacc_pool.tile([128, 512], F32)
                        for ki in range(KCH):
                            nc.tensor.matmul(
                                pacc[:, 0:tt],
                                W_sb[:, ki * ATTN + ac * 128 : ki * ATTN + (ac + 1) * 128],
                                encT[:, ki * 512 : ki * 512 + tt],
                                start=(ki == 0),
                                stop=False,
                            )
                        nc.tensor.matmul(
                            pacc[:, 0:tt],
                            M_sb[:, ac * 128 : (ac + 1) * 128],
                            H[:, b * T + t0 : b * T + t0 + tt],
                            start=False,
                            stop=True,
                        )
                        th = th_pool.tile([128, 512], F32R)
                        nc.scalar.activation(
                            th[:, 0:tt],
                            pacc[:, 0:tt],
                            AF.Tanh,
                            bias=decbe[:, ac * BPC + b : ac * BPC + b + 1],
                        )
                        nc.tensor.matmul(
                            pe_ps[0:1, 0:tt],
                            ws_sb[:, ac : ac + 1],
                            th[:, 0:tt],
                            start=(ac == 0),
                            stop=(ac == ACH - 1),
                        )
                    e_b = eb_pool.tile([1, 512], F32)
                    nc.scalar.activation(e_b[0:1, 0:tt], pe_ps[0:1, 0:tt], AF.Exp)
                    nc.sync.dma_start(e_scr[b, t0 : t0 + tt], e_b[0:1, 0:tt])

            # ---- softmax tail (all batches at once, via DRAM bounce) ----
            if do_tail:
                tail_body(nc, tail_pool, e_scr, msk_sb, out)

    nc.compile()
    return nc


def tail_body(nc, tail_pool, e_scr, msk_sb, out):
            e4 = tail_pool.tile([BPC, T], F32)
            nc.sync.dma_start(e4[:, :], e_scr[:, :])
            e4m = tail_pool.tile([BPC, T], F32)
            s4 = tail_pool.tile([BPC, 1], F32)
            r4 = tail_pool.tile([BPC, 1], F32)
            a4 = tail_pool.tile([BPC, T], F32)
            nc.vector.tensor_mul(e4m[:, :], e4[:, :], msk_sb[:, :])
            nc.vector.reduce_sum(s4[:, 0:1], e4m[:, :], axis=mybir.AxisListType.X)
            nc.vector.reciprocal(r4[:, 0:1], s4[:, 0:1])
            nc.vector.tensor_scalar_mul(a4[:, :], e4m[:, :], r4[:, 0:1])
            nc.sync.dma_start(out[:, :], a4[:, :])


_NC_CACHE = None


def get_nc():
    global _NC_CACHE
    if _NC_CACHE is None:
        _NC_CACHE = build_nc()
    return _NC_CACHE


def make_in_maps(enc_output, prev_dec_hidden, prev_alpha, mask,
                 W_conv, W_c2s, W_enc, b_enc, W_dec, w_score):
    enc_output = np.ascontiguousarray(np.asarray(enc_output, np.float32))
    h = np.asarray(prev_dec_hidden, np.float32)
    pa = np.asarray(prev_alpha, np.float32)
    mask = np.ascontiguousarray(np.asarray(mask, np.float32))

    apad = np.zeros((B, TP), np.float32)
    apad[:, PAD : PAD + T] = pa[:, 0, :]

    wconv = np.ascontiguousarray(np.asarray(W_conv, np.float32).reshape(NK, KW))
    wc2s = np.ascontiguousarray(np.asarray(W_c2s, np.float32))
    wenc = np.ascontiguousarray(np.asarray(W_enc, np.float32))
    wdec = np.ascontiguousarray(np.asarray(W_dec, np.float32))
    wsc = np.ascontiguousarray(
        np.asarray(w_score, np.float32).reshape(ACH, 128).T
    )
    bencT = np.ascontiguousarray(
        np.asarray(b_enc, np.float32).reshape(ACH, 128).T
    )
    ident = np.eye(128, dtype=np.float32)

    in_maps = []
    for c in range(NCORES):
        s = slice(c * BPC, (c + 1) * BPC)
        in_maps.append(
            {
                "enc": np.ascontiguousarray(enc_output[s]),
                "apad": np.ascontiguousarray(apad[s]),
                "mask": np.ascontiguousarray(mask[s]),
                "hT": np.ascontiguousarray(h[s].T),
                "wconv": wconv,
                "wc2s": wc2s,
                "wenc": wenc,
                "bencT": bencT,
                "wdec": wdec,
                "wsc": wsc,
                "ident": ident,
            }
        )
    return in_maps


def kernel(**inputs) -> np.ndarray:
    from concourse.bass_utils import run_bass_kernel_spmd

    nc = get_nc()
    in_maps = make_in_maps(**inputs)
    res = run_bass_kernel_spmd(nc, in_maps, core_ids=list(range(NCORES)))
    outs = [np.asarray(res.results[c]["out"]) for c in range(NCORES)]
    alpha = np.concatenate(outs, axis=0).reshape(B, 1, T).astype(np.float32)
    return alpha


# revision 9
# speedup vs baseline: 1.0958x; 1.0958x over previous
"""Trainium2 Bass kernel for location-sensitive attention.

alpha = softmax(w_score . tanh(enc @ W_enc + b_enc + h @ W_dec + conv(prev_alpha) @ W_c2s)) * mask

Sharding: data-parallel over batch B=32 across 8 cores (4 batches/core).
All weights replicated. Full inputs in, full output out.

Per-core dataflow (T=2000, K=1024, A=512, batches=4):
  - enc tiles DMA'd naturally as [t<=128, 1024] (contiguous rows).
  - TensorE transpose-mode flips each [t,128k] block into PSUM as [128k, t];
    ACT/DVE copy assembles encT [128k, t] in SBUF.
  - Regular matmuls (float32r: full-rate fp32 data) accumulate in PSUM [a128, t512]:
      8 chunks of W_enc.T-contraction + 1 conv matmul (Hankel view of padded
      alpha against M = W_conv.T @ W_c2s, a rank-100 contraction).
  - ACT applies tanh PSUM->SBUF with per-partition bias = dec_e[b] + b_enc
    (computed transposed on-device); TensorE contracts with w_score into
    PSUM e[1, t]; ACT applies exp (softmax max-subtraction is skipped:
    |e| <= ||w_score||_1 ~ 16, safely inside fp32 exp range; alpha invariant).
  - Rows packed [4, T] via a DRAM bounce; DVE masked sum + reciprocal + scale.
"""

import os
import sys
import numpy as np
import dataclasses

for _p in ("/opt/trn_rl_repo", "/root/.axon_site/_ro/trn_rl_repo"):
    if os.path.isdir(_p) and _p not in sys.path:
        sys.path.append(_p)

import concourse.bass as bass
import concourse.bacc as bacc
import concourse.mybir as mybir
from concourse import tile

B, T, ENC2, DEC, ATTN = 32, 2000, 1024, 512, 512
NK, KW, PAD = 10, 100, 50
NCORES = 8
BPC = B // NCORES  # batches per core
TP = T + KW  # padded alpha length (50 + 2000 + 50)

F32 = mybir.dt.float32
F32R = mybir.dt.float32r
BF16 = mybir.dt.bfloat16
AF = mybir.ActivationFunctionType

KCH = ENC2 // 128  # 8 contraction chunks
ACH = ATTN // 128  # 4 a-chunks
T_TILES = [(0, 512), (512, 512), (1024, 512), (1536, 464)]


def _subchunks(tt):
    subs = []
    j0 = 0
    while j0 < tt:
        subs.append((j0, min(128, tt - j0)))
        j0 += 128
    return subs


def r32(ap):
    return ap.bitcast(F32R)


def build_nc(n_batches=None, n_ttiles=None, do_main=True, do_tail=True):
    if n_batches is None:
        n_batches = int(os.environ.get("K_BATCHES", BPC))
    if n_ttiles is None:
        n_ttiles = int(os.environ.get("K_TTILES", len(T_TILES)))
    do_main = do_main and os.environ.get("K_MAIN", "1") == "1"
    do_tail = do_tail and os.environ.get("K_TAIL", "1") == "1"
    nc = bacc.Bacc(None, target_bir_lowering=False)

    enc = nc.declare_dram_parameter("enc", [BPC, T, ENC2], F32, isOutput=False)
    apad = nc.declare_dram_parameter("apad", [BPC, TP], F32, isOutput=False)
    mask = nc.declare_dram_parameter("mask", [BPC, T], F32, isOutput=False)
    ht = nc.declare_dram_parameter("hT", [DEC, BPC], F32, isOutput=False)
    wconv = nc.declare_dram_parameter("wconv", [NK, KW], F32, isOutput=False)
    wc2s = nc.declare_dram_parameter("wc2s", [NK, ATTN], F32, isOutput=False)
    wenc = nc.declare_dram_parameter("wenc", [ENC2, ATTN], F32, isOutput=False)
    bencT = nc.declare_dram_parameter("bencT", [128, ACH], F32, isOutput=False)
    wdec = nc.declare_dram_parameter("wdec", [DEC, ATTN], F32, isOutput=False)
    wsc = nc.declare_dram_parameter("wsc", [128, ACH], F32, isOutput=False)
    ident = nc.declare_dram_parameter("ident", [128, 128], F32, isOutput=False)
    out = nc.declare_dram_parameter("out", [BPC, T], F32, isOutput=True)

    e_scr = nc.dram_tensor("e_scr", [BPC, T], F32)

    with tile.TileContext(nc) as tc:
        with (
            tc.tile_pool(name="const", bufs=1) as cpool,
            tc.tile_pool(name="nat", bufs=8) as nat_pool,
            tc.tile_pool(name="encT", bufs=2) as encT_pool,
            tc.tile_pool(name="th", bufs=4) as th_pool,
            tc.tile_pool(name="eb", bufs=2) as eb_pool,
            tc.tile_pool(name="tail", bufs=1) as tail_pool,
            tc.tile_pool(name="ptr", bufs=3, space="PSUM") as ptr_pool,
            tc.tile_pool(name="pacc", bufs=2, space="PSUM") as pacc_pool,
            tc.tile_pool(name="pe", bufs=2, space="PSUM") as pe_pool,
        ):
            # ---- constants / weights into SBUF ----
            W_f = cpool.tile([128, KCH * ATTN], F32)  # [128, 4096]
            for ki in range(KCH):
                nc.sync.dma_start(
                    W_f[:, ki * ATTN : (ki + 1) * ATTN],
                    wenc[ki * 128 : (ki + 1) * 128, :],
                )
            W_sb = cpool.tile([128, KCH * ATTN], BF16)
            nc.vector.tensor_copy(W_sb[:, :], W_f[:, :])
            wc_sb = cpool.tile([NK, KW], F32)
            nc.sync.dma_start(wc_sb[:, :], wconv[:, :])
            wcs_sb = cpool.tile([NK, ATTN], F32)
            nc.sync.dma_start(wcs_sb[:, :], wc2s[:, :])
            ht_sb = cpool.tile([128, 4 * BPC], F32)
            for c in range(4):
                nc.sync.dma_start(
                    ht_sb[:, c * BPC : (c + 1) * BPC],
                    ht[c * 128 : (c + 1) * 128, :],
                )
            wd_sb = cpool.tile([128, 4 * ATTN], F32)
            for c in range(4):
                nc.sync.dma_start(
                    wd_sb[:, c * ATTN : (c + 1) * ATTN],
                    wdec[c * 128 : (c + 1) * 128, :],
                )
            be_sb = cpool.tile([128, ACH], F32)
            nc.sync.dma_start(be_sb[:, :], bencT[:, :])
            msk_sb = cpool.tile([BPC, T], F32)
            nc.sync.dma_start(msk_sb[:, :], mask[:, :])
            ws_sb = cpool.tile([128, ACH], BF16)
            nc.gpsimd.dma_start(ws_sb[:, :], wsc[:, :])
            id_sb = cpool.tile([128, 128], F32)
            nc.sync.dma_start(id_sb[:, :], ident[:, :])
            id_bf = cpool.tile([128, 128], BF16)
            nc.vector.tensor_copy(id_bf[:, :], id_sb[:, :])

            # M = wconv.T @ wc2s  [100, 512] ; decbe [128, ACH*BPC]:
            #   decbe[p, ac*BPC+b] = sum_d h[b,d] wdec[d, ac*128+p] + b_enc[ac*128+p]
            M_sb = cpool.tile([KW, ATTN], BF16)
            decbe = cpool.tile([128, ACH * BPC], F32)
            # H: [100, BPC*2000] Hankel(alpha_pad)
            H = cpool.tile([KW, BPC * T], BF16)

            with tc.tile_pool(name="psetup", bufs=1, space="PSUM") as spool:
                m_ps = spool.tile([KW, ATTN], F32, tag="s")
                nc.tensor.matmul(
                    m_ps[:, :], wc_sb[:, :], wcs_sb[:, :], start=True, stop=True
                )
                nc.scalar.copy(M_sb[:, :], m_ps[:, :])
                dec_ps = spool.tile([128, ACH * BPC], F32, tag="s")
                for ac in range(ACH):
                    for c in range(4):
                        nc.tensor.matmul(
                            dec_ps[:, ac * BPC : (ac + 1) * BPC],
                            wd_sb[:, c * ATTN + ac * 128 : c * ATTN + (ac + 1) * 128],
                            ht_sb[:, c * BPC : (c + 1) * BPC],
                            start=(c == 0),
                            stop=(c == 3),
                        )
                for ac in range(ACH):
                    nc.scalar.activation(
                        decbe[:, ac * BPC : (ac + 1) * BPC],
                        dec_ps[:, ac * BPC : (ac + 1) * BPC],
                        AF.Identity,
                        bias=be_sb[:, ac : ac + 1],
                    )

            for b in range(BPC):
                hank = dataclasses.replace(
                    apad[b : b + 1, :], ap=[[1, KW], [1, T]]
                )
                nc.gpsimd.dma_start(H[0:KW, b * T : (b + 1) * T], hank)

            # ---- main loop ----
            for b in range(n_batches if do_main else 0):
                for t0, tt in T_TILES[:n_ttiles]:
                    subs = _subchunks(tt)
                    nats = []
                    idx_eng = b * 4
                    for j0, tj in subs:
                        natt = nat_pool.tile([128, ENC2], F32)
                        nc.sync.dma_start(
                            natt[0:tj, :], enc[b, t0 + j0 : t0 + j0 + tj, :]
                        )
                        natb = nat_pool.tile([128, ENC2], BF16, tag="natb")
                        if idx_eng % 2 == 0:
                            nc.vector.tensor_copy(natb[0:tj, :], natt[0:tj, :])
                        else:
                            nc.scalar.copy(natb[0:tj, :], natt[0:tj, :])
                        idx_eng += 1
                        nats.append(natb)
                    encT = encT_pool.tile([128, KCH * 512], BF16)
                    for ki in range(KCH):
                        ptr = ptr_pool.tile([128, 512], BF16)
                        for idx, (j0, tj) in enumerate(subs):
                            nc.tensor.transpose(
                                ptr[:, j0 : j0 + tj],
                                nats[idx][0:tj, ki * 128 : (ki + 1) * 128],
                                id_bf[0:tj, 0:tj],
                            )
                        if ki % 2 == 0:
                            nc.scalar.copy(
                                encT[:, ki * 512 : ki * 512 + tt], ptr[:, 0:tt]
                            )
                        else:
                            nc.vector.tensor_copy(
                                encT[:, ki * 512 : ki * 512 + tt], ptr[:, 0:tt]
                            )
                    pe_ps = pe_pool.tile([1, 512], F32)
                    for ac in range(ACH):
                        pacc = pacc_pool.tile([128, 512], F32)
                        for ki in range(KCH):
                            nc.tensor.matmul(
                                pacc[:, 0:tt],
                                W_sb[:, ki * ATTN + ac * 128 : ki * ATTN + (ac + 1) * 128],
                                encT[:, ki * 512 : ki * 512 + tt],
                                start=(ki == 0),
                                stop=False,
                            )
                        nc.tensor.matmul(
                            pacc[:, 0:tt],
                            M_sb[:, ac * 128 : (ac + 1) * 128],
                            H[:, b * T + t0 : b * T + t0 + tt],
                            start=False,
                            stop=True,
                        )
                        th = th_pool.tile([128, 512], BF16)
                        nc.scalar.activation(
                            th[:, 0:tt],
                            pacc[:, 0:tt],
                            AF.Tanh,
                            bias=decbe[:, ac * BPC + b : ac * BPC + b + 1],
                        )
                        nc.tensor.matmul(
                            pe_ps[0:1, 0:tt],
                            ws_sb[:, ac : ac + 1],
                            th[:, 0:tt],
                            start=(ac == 0),
                            stop=(ac == ACH - 1),
                        )
                    e_b = eb_pool.tile([1, 512], F32)
                    nc.scalar.activation(e_b[0:1, 0:tt], pe_ps[0:1, 0:tt], AF.Exp)
                    nc.sync.dma_start(e_scr[b, t0 : t0 + tt], e_b[0:1, 0:tt])

            # ---- softmax tail (all batches at once, via DRAM bounce) ----
            if do_tail:
                tail_body(nc, tail_pool, e_scr, msk_sb, out)

    nc.compile()
    return nc


def tail_body(nc, tail_pool, e_scr, msk_sb, out):
            e4 = tail_pool.tile([BPC, T], F32)
            nc.sync.dma_start(e4[:, :], e_scr[:, :])
            e4m = tail_pool.tile([BPC, T], F32)
            s4 = tail_pool.tile([BPC, 1], F32)
            r4 = tail_pool.tile([BPC, 1], F32)
            a4 = tail_pool.tile([BPC, T], F32)
            nc.vector.tensor_mul(e4m[:, :], e4[:, :], msk_sb[:, :])
            nc.vector.reduce_sum(s4[:, 0:1], e4m[:, :], axis=mybir.AxisListType.X)
            nc.vector.reciprocal(r4[:, 0:1], s4[:, 0:1])
            nc.vector.tensor_scalar_mul(a4[:, :], e4m[:, :], r4[:, 0:1])
            nc.sync.dma_start(out[:, :], a4[:, :])


_NC_CACHE = None


def get_nc():
    global _NC_CACHE
    if _NC_CACHE is None:
        _NC_CACHE = build_nc()
    return _NC_CACHE


def make_in_maps(enc_output, prev_dec_hidden, prev_alpha, mask,
                 W_conv, W_c2s, W_enc, b_enc, W_dec, w_score):
    enc_output = np.ascontiguousarray(np.asarray(enc_output, np.float32))
    h = np.asarray(prev_dec_hidden, np.float32)
    pa = np.asarray(prev_alpha, np.float32)
    mask = np.ascontiguousarray(np.asarray(mask, np.float32))

    apad = np.zeros((B, TP), np.float32)
    apad[:, PAD : PAD + T] = pa[:, 0, :]

    wconv = np.ascontiguousarray(np.asarray(W_conv, np.float32).reshape(NK, KW))
    wc2s = np.ascontiguousarray(np.asarray(W_c2s, np.float32))
    wenc = np.ascontiguousarray(np.asarray(W_enc, np.float32))
    wdec = np.ascontiguousarray(np.asarray(W_dec, np.float32))
    wsc = np.ascontiguousarray(
        np.asarray(w_score, np.float32).reshape(ACH, 128).T
    )
    bencT = np.ascontiguousarray(
        np.asarray(b_enc, np.float32).reshape(ACH, 128).T
    )
    ident = np.eye(128, dtype=np.float32)

    in_maps = []
    for c in range(NCORES):
        s = slice(c * BPC, (c + 1) * BPC)
        in_maps.append(
            {
                "enc": np.ascontiguousarray(enc_output[s]),
                "apad": np.ascontiguousarray(apad[s]),
                "mask": np.ascontiguousarray(mask[s]),
                "hT": np.ascontiguousarray(h[s].T),
                "wconv": wconv,
                "wc2s": wc2s,
                "wenc": wenc,
                "bencT": bencT,
                "wdec": wdec,
                "wsc": wsc,
                "ident": ident,
            }
        )
    return in_maps


def kernel(**inputs) -> np.ndarray:
    from concourse.bass_utils import run_bass_kernel_spmd

    nc = get_nc()
    in_maps = make_in_maps(**inputs)
    res = run_bass_kernel_spmd(nc, in_maps, core_ids=list(range(NCORES)))
    outs = [np.asarray(res.results[c]["out"]) for c in range(NCORES)]
    alpha = np.concatenate(outs, axis=0).reshape(B, 1, T).astype(np.float32)
    return alpha


# revision 11
# speedup vs baseline: 1.1501x; 1.0495x over previous
"""Trainium2 Bass kernel for location-sensitive attention.

alpha = softmax(w_score . tanh(enc @ W_enc + b_enc + h @ W_dec + conv(prev_alpha) @ W_c2s)) * mask

Sharding: data-parallel over batch B=32 across 8 cores (4 batches/core).
All weights replicated. Full inputs in, full output out.

Per-core dataflow (T=2000, K=1024, A=512, batches=4):
  - enc tiles DMA'd naturally as [t<=128, 1024] f32 (contiguous rows), cast
    to bf16 on DVE.
  - TensorE transpose-mode flips each [t,128k] bf16 block into PSUM;
    ACT copies assemble encT [128k, t] in SBUF.
  - bf16 matmuls accumulate in PSUM [a128, t512]: 8 chunks of W_enc.T
    contraction + 1 conv matmul (Hankel view of padded alpha against
    M = W_conv.T @ W_c2s, rank-100 contraction).
  - ACT applies tanh PSUM->SBUF(bf16) with per-partition bias
    = dec_e[b] + b_enc (computed transposed on-device); TensorE contracts
    with w_score into PSUM e[1, t]; ACT applies exp (softmax max-subtraction
    is skipped: |e| <= ||w_score||_1 ~ 16, safely inside fp32 exp range;
    alpha is invariant to the shift).
  - Per-batch tail on DVE (masked sum, reciprocal, scale), overlapped with
    the next batch's compute; direct DMA of each alpha row to the output.
"""

import os
import sys
import numpy as np
import dataclasses

for _p in ("/opt/trn_rl_repo", "/root/.axon_site/_ro/trn_rl_repo"):
    if os.path.isdir(_p) and _p not in sys.path:
        sys.path.append(_p)

import concourse.bass as bass
import concourse.bacc as bacc
import concourse.mybir as mybir
from concourse import tile

B, T, ENC2, DEC, ATTN = 32, 2000, 1024, 512, 512
NK, KW, PAD = 10, 100, 50
NCORES = 8
BPC = B // NCORES  # batches per core
TP = T + KW  # padded alpha length (50 + 2000 + 50)

F32 = mybir.dt.float32
BF16 = mybir.dt.bfloat16
AF = mybir.ActivationFunctionType

KCH = ENC2 // 128  # 8 contraction chunks
ACH = ATTN // 128  # 4 a-chunks
T_TILES = [(0, 512), (512, 512), (1024, 512), (1536, 464)]


def _subchunks(tt):
    subs = []
    j0 = 0
    while j0 < tt:
        subs.append((j0, min(128, tt - j0)))
        j0 += 128
    return subs


def build_nc():
    nc = bacc.Bacc(None, target_bir_lowering=False)

    enc = nc.declare_dram_parameter("enc", [BPC, T, ENC2], F32, isOutput=False)
    apad = nc.declare_dram_parameter("apad", [BPC, TP], F32, isOutput=False)
    mask = nc.declare_dram_parameter("mask", [BPC, T], F32, isOutput=False)
    ht = nc.declare_dram_parameter("hT", [DEC, BPC], F32, isOutput=False)
    wconv = nc.declare_dram_parameter("wconv", [NK, KW], F32, isOutput=False)
    wc2s = nc.declare_dram_parameter("wc2s", [NK, ATTN], F32, isOutput=False)
    wenc = nc.declare_dram_parameter("wenc", [ENC2, ATTN], F32, isOutput=False)
    bencT = nc.declare_dram_parameter("bencT", [128, ACH], F32, isOutput=False)
    wdec = nc.declare_dram_parameter("wdec", [DEC, ATTN], F32, isOutput=False)
    wsc = nc.declare_dram_parameter("wsc", [128, ACH], F32, isOutput=False)
    ident = nc.declare_dram_parameter("ident", [128, 128], F32, isOutput=False)
    out = nc.declare_dram_parameter("out", [BPC, T], F32, isOutput=True)

    with tile.TileContext(nc) as tc:
        with (
            tc.tile_pool(name="const", bufs=1) as cpool,
            tc.tile_pool(name="nat", bufs=12) as nat_pool,
            tc.tile_pool(name="encT", bufs=2) as encT_pool,
            tc.tile_pool(name="th", bufs=4) as th_pool,
            tc.tile_pool(name="eb", bufs=2) as eb_pool,
            tc.tile_pool(name="ptr", bufs=3, space="PSUM") as ptr_pool,
            tc.tile_pool(name="pacc", bufs=2, space="PSUM") as pacc_pool,
            tc.tile_pool(name="pe", bufs=2, space="PSUM") as pe_pool,
        ):
            # ---- prefetch: first batch's first tiles + identity before the
            # weight pack, so PE transposes can start ASAP ----
            def load_nat(b, t0, tt):
                subs = _subchunks(tt)
                nats = []
                for j0, tj in subs:
                    natt = nat_pool.tile([128, ENC2], F32, tag="natf")
                    nc.sync.dma_start(
                        natt[0:tj, :], enc[b, t0 + j0 : t0 + j0 + tj, :]
                    )
                    natb = nat_pool.tile([128, ENC2], BF16, tag="natb")
                    nc.vector.tensor_copy(natb[0:tj, :], natt[0:tj, :])
                    nats.append(natb)
                return nats

            id_sb = cpool.tile([128, 128], F32)
            nc.sync.dma_start(id_sb[:, :], ident[:, :])
            id_bf = cpool.tile([128, 128], BF16)
            nc.vector.tensor_copy(id_bf[:, :], id_sb[:, :])

            prefetched = {}
            for ti in (0, 1):
                t0, tt = T_TILES[ti]
                prefetched[(0, ti)] = load_nat(0, t0, tt)

            # ---- constants / weights into SBUF ----
            W_f = cpool.tile([128, KCH * ATTN], F32)  # [128, 4096]
            for ki in range(KCH):
                nc.sync.dma_start(
                    W_f[:, ki * ATTN : (ki + 1) * ATTN],
                    wenc[ki * 128 : (ki + 1) * 128, :],
                )
            W_sb = cpool.tile([128, KCH * ATTN], BF16)
            nc.vector.tensor_copy(W_sb[:, :], W_f[:, :])
            wc_sb = cpool.tile([NK, KW], F32)
            nc.sync.dma_start(wc_sb[:, :], wconv[:, :])
            wcs_sb = cpool.tile([NK, ATTN], F32)
            nc.sync.dma_start(wcs_sb[:, :], wc2s[:, :])
            ht_sb = cpool.tile([128, 4 * BPC], F32)
            for c in range(4):
                nc.sync.dma_start(
                    ht_sb[:, c * BPC : (c + 1) * BPC],
                    ht[c * 128 : (c + 1) * 128, :],
                )
            wd_sb = cpool.tile([128, 4 * ATTN], F32)
            for c in range(4):
                nc.sync.dma_start(
                    wd_sb[:, c * ATTN : (c + 1) * ATTN],
                    wdec[c * 128 : (c + 1) * 128, :],
                )
            be_sb = cpool.tile([128, ACH], F32)
            nc.sync.dma_start(be_sb[:, :], bencT[:, :])
            ws_sb = cpool.tile([128, ACH], BF16)
            nc.gpsimd.dma_start(ws_sb[:, :], wsc[:, :])

            # M = wconv.T @ wc2s  [100, 512] ; decbe [128, ACH*BPC]:
            #   decbe[p, ac*BPC+b] = sum_d h[b,d] wdec[d, ac*128+p] + b_enc[ac*128+p]
            M_sb = cpool.tile([KW, ATTN], BF16)
            decbe = cpool.tile([128, ACH * BPC], F32)
            # H: [100, BPC*2000] Hankel(alpha_pad), bf16 via SWDGE cast
            H = cpool.tile([KW, BPC * T], BF16)

            with tc.tile_pool(name="psetup", bufs=1, space="PSUM") as spool:
                m_ps = spool.tile([KW, ATTN], F32, tag="s")
                nc.tensor.matmul(
                    m_ps[:, :], wc_sb[:, :], wcs_sb[:, :], start=True, stop=True
                )
                nc.scalar.copy(M_sb[:, :], m_ps[:, :])
                dec_ps = spool.tile([128, ACH * BPC], F32, tag="s")
                for ac in range(ACH):
                    for c in range(4):
                        nc.tensor.matmul(
                            dec_ps[:, ac * BPC : (ac + 1) * BPC],
                            wd_sb[:, c * ATTN + ac * 128 : c * ATTN + (ac + 1) * 128],
                            ht_sb[:, c * BPC : (c + 1) * BPC],
                            start=(c == 0),
                            stop=(c == 3),
                        )
                for ac in range(ACH):
                    nc.scalar.activation(
                        decbe[:, ac * BPC : (ac + 1) * BPC],
                        dec_ps[:, ac * BPC : (ac + 1) * BPC],
                        AF.Identity,
                        bias=be_sb[:, ac : ac + 1],
                    )

            for b in range(BPC):
                hank = dataclasses.replace(
                    apad[b : b + 1, :], ap=[[1, KW], [1, T]]
                )
                nc.gpsimd.dma_start(H[0:KW, b * T : (b + 1) * T], hank)

            # ---- main loop ----
            def emit_tail(b, e_b, mskb):
                em = eb_pool.tile([1, T], F32, tag="em")
                s1 = eb_pool.tile([1, 1], F32, tag="s1")
                r1 = eb_pool.tile([1, 1], F32, tag="r1")
                a1 = eb_pool.tile([1, T], F32, tag="a1")
                nc.vector.tensor_mul(em[0:1, :], e_b[0:1, :], mskb[0:1, :])
                nc.vector.reduce_sum(
                    s1[0:1, 0:1], em[0:1, :], axis=mybir.AxisListType.X
                )
                nc.vector.reciprocal(r1[0:1, 0:1], s1[0:1, 0:1])
                nc.vector.tensor_scalar_mul(a1[0:1, :], em[0:1, :], r1[0:1, 0:1])
                nc.sync.dma_start(out[b : b + 1, :], a1[0:1, :])

            pending_tail = None
            for b in range(BPC):
                e_b = eb_pool.tile([1, T], F32, tag="e_b")
                mskb = eb_pool.tile([1, T], F32, tag="mskb")
                nc.sync.dma_start(mskb[0:1, :], mask[b : b + 1, :])
                for ti, (t0, tt) in enumerate(T_TILES):
                    nats = prefetched.pop((b, ti), None)
                    if nats is None:
                        nats = load_nat(b, t0, tt)
                    # prior batch's tail goes to DVE after this tile's casts
                    if pending_tail is not None and ti == 1:
                        emit_tail(*pending_tail)
                        pending_tail = None
                    subs = _subchunks(tt)
                    encT = encT_pool.tile([128, KCH * 512], BF16)
                    for ki in range(KCH):
                        ptr = ptr_pool.tile([128, 512], BF16)
                        for idx, (j0, tj) in enumerate(subs):
                            nc.tensor.transpose(
                                ptr[:, j0 : j0 + tj],
                                nats[idx][0:tj, ki * 128 : (ki + 1) * 128],
                                id_bf[0:tj, 0:tj],
                            )
                        nc.scalar.copy(
                            encT[:, ki * 512 : ki * 512 + tt], ptr[:, 0:tt]
                        )
                    pe_ps = pe_pool.tile([1, 512], F32)
                    for ac in range(ACH):
                        pacc = pacc_pool.tile([128, 512], F32)
                        for ki in range(KCH):
                            nc.tensor.matmul(
                                pacc[:, 0:tt],
                                W_sb[:, ki * ATTN + ac * 128 : ki * ATTN + (ac + 1) * 128],
                                encT[:, ki * 512 : ki * 512 + tt],
                                start=(ki == 0),
                                stop=False,
                            )
                        nc.tensor.matmul(
                            pacc[:, 0:tt],
                            M_sb[:, ac * 128 : (ac + 1) * 128],
                            H[:, b * T + t0 : b * T + t0 + tt],
                            start=False,
                            stop=True,
                        )
                        th = th_pool.tile([128, 512], BF16)
                        nc.scalar.activation(
                            th[:, 0:tt],
                            pacc[:, 0:tt],
                            AF.Tanh,
                            bias=decbe[:, ac * BPC + b : ac * BPC + b + 1],
                        )
                        nc.tensor.matmul(
                            pe_ps[0:1, 0:tt],
                            ws_sb[:, ac : ac + 1],
                            th[:, 0:tt],
                            start=(ac == 0),
                            stop=(ac == ACH - 1),
                        )
                    nc.scalar.activation(
                        e_b[0:1, t0 : t0 + tt], pe_ps[0:1, 0:tt], AF.Exp
                    )
                    # prefetch next tiles' enc data
                    nxt = (b, ti + 2) if ti + 2 < len(T_TILES) else (b + 1, ti - 2)
                    if nxt[0] < BPC and (nxt not in prefetched):
                        nt0, ntt = T_TILES[nxt[1]]
                        prefetched[nxt] = load_nat(nxt[0], nt0, ntt)
                pending_tail = (b, e_b, mskb)
            if pending_tail is not None:
                emit_tail(*pending_tail)

    nc.compile()
    return nc


_NC_CACHE = None


def get_nc():
    global _NC_CACHE
    if _NC_CACHE is None:
        _NC_CACHE = build_nc()
    return _NC_CACHE


def make_in_maps(enc_output, prev_dec_hidden, prev_alpha, mask,
                 W_conv, W_c2s, W_enc, b_enc, W_dec, w_score):
    enc_output = np.ascontiguousarray(np.asarray(enc_output, np.float32))
    h = np.asarray(prev_dec_hidden, np.float32)
    pa = np.asarray(prev_alpha, np.float32)
    mask = np.ascontiguousarray(np.asarray(mask, np.float32))

    apad = np.zeros((B, TP), np.float32)
    apad[:, PAD : PAD + T] = pa[:, 0, :]

    wconv = np.ascontiguousarray(np.asarray(W_conv, np.float32).reshape(NK, KW))
    wc2s = np.ascontiguousarray(np.asarray(W_c2s, np.float32))
    wenc = np.ascontiguousarray(np.asarray(W_enc, np.float32))
    wdec = np.ascontiguousarray(np.asarray(W_dec, np.float32))
    wsc = np.ascontiguousarray(
        np.asarray(w_score, np.float32).reshape(ACH, 128).T
    )
    bencT = np.ascontiguousarray(
        np.asarray(b_enc, np.float32).reshape(ACH, 128).T
    )
    ident = np.eye(128, dtype=np.float32)

    in_maps = []
    for c in range(NCORES):
        s = slice(c * BPC, (c + 1) * BPC)
        in_maps.append(
            {
                "enc": np.ascontiguousarray(enc_output[s]),
                "apad": np.ascontiguousarray(apad[s]),
                "mask": np.ascontiguousarray(mask[s]),
                "hT": np.ascontiguousarray(h[s].T),
                "wconv": wconv,
                "wc2s": wc2s,
                "wenc": wenc,
                "bencT": bencT,
                "wdec": wdec,
                "wsc": wsc,
                "ident": ident,
            }
        )
    return in_maps


def kernel(**inputs) -> np.ndarray:
    from concourse.bass_utils import run_bass_kernel_spmd

    nc = get_nc()
    in_maps = make_in_maps(**inputs)
    res = run_bass_kernel_spmd(nc, in_maps, core_ids=list(range(NCORES)))
    outs = [np.asarray(res.results[c]["out"]) for c in range(NCORES)]
    alpha = np.concatenate(outs, axis=0).reshape(B, 1, T).astype(np.float32)
    return alpha
